# revision 51
# baseline (speedup 1.0000x reference)
"""Trainium2 Bass kernel for nn_CausalCosmosBlock (sink+sliding-window attention block).

Reference computation (B=2, L=256, D=4096, H=32, Dh=128, SINK=128, S=4224):
    q = rmsnorm((x @ Wq).reshape(B,L,H,Dh)) ; k likewise ; v = x @ Wv
    new_k = concat(cache_k[:, :128], cache_k[:, 384:], k)   # S rows
    logits = q @ new_k^T / sqrt(Dh), causal mask j <= (S-L)+i
    out = softmax(logits) @ new_v ; return out @ Wo

Sharding: tensor-parallel over heads. 8 cores x 4 heads. Each core computes
its heads' projections from the full x, attention over its heads' cache, and
a partial y = out_heads @ Wo[head_rows]; host sums the 8 partials.

Device layout choices (everything chosen so no on-device transposes needed):
  - x is fed transposed (xT [D, B*L]); projections produce qT/kT [Dh, tok].
  - K-cache fed pre-transposed per head ([Dh, S_keep]); logits computed as
    logits^T [s, l] chunks so softmax sums reduce over partitions via a
    ones-vector matmul, and attn@V uses V in natural [s, d] layout as lhsT.
  - V-cache fed as [S_keep, 4*Dh] (head-interleaved); new-chunk V computed
    token-major directly.
All matmul operands fp16 (1 cycle/row on PE); accumulation fp32 in PSUM;
softmax stats fp32; output partials stored fp16 and host-summed in fp64.

Schedule (measured on HW; the kernel is PE-bound at ~171us of matmul
cycles, DMA ~131us rides the shared ~315GB/s per-core HBM pipe):
  - Q-proj | K-proj | V-proj(b0 tokens) on PE, with each projection's
    rmsnorm ss/winv matmuls deferred behind the NEXT projection's matmuls
    so the PE's in-order FIFO never head-of-line blocks on DVE/ACT.
  - attention b0 with V-proj(b1 tokens) drip-fed into the ACT-bound
    stretches; attention b1 with b0's output projection interleaved;
    b1's output projection as the tail.
  - attn@V matmuls are emitted one chunk-group late so exp() has always
    completed when they reach the PE FIFO head (A/B: ~35us).
  - softmax denominator: DVE lane-adds -> PE ones-fold into the spare
    half of the head's oacc PSUM bank -> DVE reciprocal_approx_fast ->
    PE broadcast -> DVE normalize mul (no ACT Reciprocal: the ACT engine
    runs Exp-only with a single act-table load, prewarmed in the proj
    phase).
  - DMA: weights/x/k-cache/y-stores on the SP HWDGE ring, v-cache/wo on
    the Pool SWDGE ring gated behind K-proj h0 (keeps the early HBM pipe
    exclusive to the PE-critical weight stream); y fp16.
"""

import contextlib

import numpy as np

import concourse.bass as bass
import concourse.tile as tile
import concourse.mybir as mybir
from concourse import bass_isa
from concourse import bacc
from concourse.bass_utils import run_bass_kernel_spmd

# Problem shapes (hardcoded per contract)
B = 2
L = 256
D = 4096
H = 32
Dh = 128
SINK = 128
S = 4224
KEEP = SINK + (S - SINK - L)  # 3968 old cache rows kept
ST = KEEP // 128  # 31 old s-tiles of 128
KT = D // 128  # 32 contraction tiles
TOK = B * L  # 512
N_CORES = 8
HPC = H // N_CORES  # 4 heads per core
EPS = 1e-6
SCALE = 1.0 / float(np.sqrt(Dh))

DT = mybir.dt.float16
DT_NP = np.float16
F32 = mybir.dt.float32
F32R = mybir.dt.float32r

GRP = 4  # row-blocks per batched DMA

# Diagnostic ablations (timing experiments only; all False for real runs)
DIAG_NO_SUMS = False
DIAG_NO_ATTN = False
DIAG_NO_QKV = False

# Optimization toggles (for HW A/B bisection)
OPT_GATE = True  # gate Pool-ring cache DMAs behind K-proj h0
OPT_ACC4 = True  # 4-lane exp accumulator (two independent DVE add chains)
OPT_DELAY_ATTNV = True  # emit attn@V one group late (avoid PE FIFO block)


def _declare_io(nc, with_reps=False):
    t = {}
    t["xT"] = nc.dram_tensor("xT", [D, TOK], DT, kind="ExternalInput")
    t["wq"] = nc.dram_tensor("wq", [D, HPC * Dh], DT, kind="ExternalInput")
    t["wk"] = nc.dram_tensor("wk", [D, HPC * Dh], DT, kind="ExternalInput")
    t["wv"] = nc.dram_tensor("wv", [D, HPC * Dh], DT, kind="ExternalInput")
    # wo[p, h, d] = Wo[head_base + h*128 + p, d]
    t["wo"] = nc.dram_tensor("wo", [128, HPC, D], DT, kind="ExternalInput")
    t["kTc"] = nc.dram_tensor("kTc", [B, HPC, Dh, KEEP], DT, kind="ExternalInput")
    t["vc"] = nc.dram_tensor("vc", [B, KEEP, HPC * Dh], DT, kind="ExternalInput")
    t["qw"] = nc.dram_tensor("qw", [1, Dh], F32, kind="ExternalInput")
    t["kw"] = nc.dram_tensor("kw", [1, Dh], F32, kind="ExternalInput")
    t["maskt"] = nc.dram_tensor("maskt", [128, 2, L], DT, kind="ExternalInput")
    if with_reps:
        t["reps"] = nc.dram_tensor("reps", [1, 1], mybir.dt.uint32, kind="ExternalInput")
    t["yT"] = nc.dram_tensor("yT", [D, TOK], DT, kind="ExternalOutput")
    return t


def _grouped_rows(ap2d, grp=GRP):
    """View a [n*grp*128, width] DRAM AP as [g][128(p), j, width] row groups."""
    return ap2d.rearrange("(g j p) f -> g p j f", p=128, j=grp)


def _scalar_act_raw(nc, out, in_, func, scale=1.0, bias=0.0):
    """Activation-engine op emitted directly, bypassing bass's guard on
    Reciprocal/Rsqrt (table approximations; plenty for this kernel's 2e-2
    gate and ~10x faster than nc.vector.reciprocal for few-partition tiles).
    `bias` may be a [P,1] SBUF AP for funcs that require an AP bias."""
    q = nc.scalar
    ins = [q.lower_ap(in_)]
    for arg in (bias, scale, 0.0):  # bias, scale, alpha
        if isinstance(arg, (int, float)):
            ins.append(mybir.ImmediateValue(dtype=mybir.dt.float32, value=float(arg)))
        else:
            ins.append(q.lower_ap(arg))
    return q.add_instruction(
        mybir.InstActivation(
            name=q.bass.get_next_instruction_name(),
            func=func,
            ins=ins,
            outs=[q.lower_ap(out)],
        )
    )


def _emit_body(nc, tc, t, consts, pools):
    """Emit one full forward pass. `consts` holds preloaded tiny const tiles."""
    qw_sb, kw_sb, mask_sb, ones_col16, ones_row16, eps_sb = consts

    # ---- batched resident loads (SP ring), interleaved with first W slabs ----
    xT_g = _grouped_rows(t["xT"])  # [8][128, 4, TOK]
    xt_groups = []

    def _load_xt_group(g):
        xg = pools["xt"].tile([128, GRP, TOK], DT, tag=f"xt{g}", name=f"xt{g}")
        nc.sync.dma_start(xg[:], xT_g[g])
        xt_groups.append(xg)

    def xt_tile(kt):
        return xt_groups[kt // GRP][:, kt % GRP, :]

    wo_sb = pools["wo"].tile([128, HPC, D], DT, tag="wo", name="wo_sb")

    # ---------------- Phase B: projections ----------------
    qn_sb = [None] * HPC
    kn_sb = [None] * HPC
    with (
        tc.tile_pool(name="pp_psum", bufs=5, space="PSUM") as pp_psum,
        tc.tile_pool(name="ss_psum", bufs=1, space="PSUM") as ss_psum,
        tc.tile_pool(name="winv_psum", bufs=2, space="PSUM") as winv_psum,
        tc.tile_pool(name="wslab", bufs=4) as wslab_pool,
        tc.tile_pool(name="praw", bufs=3) as praw_pool,
        tc.tile_pool(name="pstat", bufs=4) as pstat_pool,
    ):
        qkv_sb = pools["qkv"]
        vnew_sb = qkv_sb.tile([128, 2 * B, HPC * Dh], DT, tag="vnew", name="vnew")

        def _emit_rmsnorm(proj_i, normw, ps):
            """Per-head rmsnorm over Dh (= partitions) via ones-matmul
            reduction. DVE squares are emitted immediately (they only wait on
            the proj PSUM); the PE matmuls (ss/winv) are returned as a
            deferred closure so they land in the PE FIFO BEHIND the next
            projection's matmuls — by then the DVE work is long done and the
            PE never head-of-line blocks."""
            raws, sqs = [], []
            for h in range(HPC):
                raw = praw_pool.tile([128, TOK], DT, tag="raw", name=f"raw{proj_i}{h}")
                nc.vector.tensor_copy(raw[:], ps[h][:])
                sq = praw_pool.tile([128, TOK], DT, tag="sq", name=f"sq{proj_i}{h}")
                nc.vector.tensor_mul(sq[:], raw[:], raw[:])
                raws.append(raw)
                sqs.append(sq)

            def _pe_part():
                for h in range(HPC):
                    ss_ps = ss_psum.tile([1, TOK], F32, tag="ss", name=f"ss{proj_i}{h}")
                    nc.tensor.matmul(ss_ps[:], ones_col16[:], sqs[h][:], start=True, stop=True)
                    # inv = 1/sqrt(ss/Dh + eps) (fused Rsqrt on ACT)
                    inv = pstat_pool.tile([1, TOK], DT, tag="inv", name=f"inv{proj_i}{h}")
                    _scalar_act_raw(
                        nc,
                        inv[:],
                        ss_ps[:],
                        mybir.ActivationFunctionType.Rsqrt,
                        scale=1.0 / Dh,
                        bias=eps_sb[0:1, 0:1],
                    )
                    # winv[d, t] = norm_w[d] * inv[t] (rank-1 outer on PE)
                    winv_ps = winv_psum.tile([128, TOK], F32, tag="winv", name=f"wi{proj_i}{h}")
                    nc.tensor.matmul(winv_ps[:], normw[:], inv[:], start=True, stop=True)
                    normed = qkv_sb.tile([128, TOK], DT, tag=f"qk{proj_i}{h}", name=f"qk{proj_i}{h}")
                    nc.vector.tensor_mul(normed[:], raws[h][:], winv_ps[:])
                    if proj_i == 0:
                        qn_sb[h] = normed
                    else:
                        kn_sb[h] = normed

            return _pe_part

        deferred_rms = None
        for proj_i, (wt, normw) in enumerate(((t["wq"], qw_sb), (t["wk"], kw_sb))):
            wt_g = _grouped_rows(wt)
            ps = [pp_psum.tile([128, TOK], F32, tag="pp", name=f"pp{_h}") for _h in range(HPC)]
            for g in range(KT // GRP):
                slab = wslab_pool.tile([128, GRP, HPC * Dh], DT, tag="w", name=f"w{proj_i}_{g}")
                nc.sync.dma_start(slab[:], wt_g[g])
                if proj_i == 0 and len(xt_groups) < KT // GRP:
                    _load_xt_group(len(xt_groups))
                if DIAG_NO_QKV:
                    continue
                for j in range(GRP):
                    kt = g * GRP + j
                    for h in range(HPC):
                        nc.tensor.matmul(
                            ps[h][:],
                            slab[:, j, h * Dh : (h + 1) * Dh],
                            xt_tile(kt),
                            start=(kt == 0),
                            stop=(kt == KT - 1),
                        )
                if g == 1 and deferred_rms is not None:
                    deferred_rms()  # Q's ss/winv land behind K's first groups
                    deferred_rms = None
            if DIAG_NO_QKV:
                for h in range(HPC):
                    normed = qkv_sb.tile([128, TOK], DT, tag=f"qk{proj_i}{h}", name=f"qk{proj_i}{h}")
                    nc.vector.memset(normed[:], 0.01)
                    if proj_i == 0:
                        qn_sb[h] = normed
                    else:
                        kn_sb[h] = normed
                continue
            deferred_rms = _emit_rmsnorm(proj_i, normw, ps)

        # V projection for batch-0 tokens (c=0,1); batch-1 tokens are
        # projected later, interleaved into b0's attention (fills PE slack
        # while the ACT engine works through the exps). The wv slabs persist
        # in SBUF so the interleaved half re-reads them without re-streaming.
        wv_g = _grouped_rows(t["wv"])
        wv_slabs = []
        vps = [pp_psum.tile([128, HPC * Dh], F32, tag="pp", name=f"vp{_c}") for _c in range(2)]
        for g in range(KT // GRP):
            slab = pools["wv"].tile([128, GRP, HPC * Dh], DT, tag="wv", name=f"wv_{g}")
            nc.sync.dma_start(slab[:], wv_g[g])
            wv_slabs.append(slab)
            if DIAG_NO_QKV:
                continue
            for j in range(GRP):
                kt = g * GRP + j
                for c in range(2):
                    nc.tensor.matmul(
                        vps[c][:],
                        xt_tile(kt)[:, c * 128 : (c + 1) * 128],
                        slab[:, j, :],
                        start=(kt == 0),
                        stop=(kt == KT - 1),
                    )
            if g == 1 and deferred_rms is not None:
                deferred_rms()  # K's ss/winv land behind V's first groups
                deferred_rms = None
        if deferred_rms is not None:
            deferred_rms()
            deferred_rms = None
        if DIAG_NO_QKV:
            nc.vector.memset(vnew_sb[:], 0.01)
        else:
            for c in range(2):
                nc.vector.tensor_copy(vnew_sb[:, c, :], vps[c][:])
        # pre-warm the Exp act table during the (ACT-idle) projection phase so
        # the first attention exp doesn't eat the ~1.3us table load
        warm = pstat_pool.tile([1, 1], DT, tag="warm", name="expwarm")
        nc.scalar.activation(
            warm[:], eps_sb[:], mybir.ActivationFunctionType.Exp, scale=1.0
        )

    # ---------------- Phase C: attention + Phase D: output projection ----------------
    GRPA = 4  # attention chunks per exp group (two PSUM banks)
    groups = []
    ci = 0
    while ci < ST:
        n = min(GRPA, ST - ci)
        groups.append(("old", tuple(range(ci, ci + n))))
        ci += n
    groups.append(("new", (0, 1)))
    assert GRPA >= 2

    VG = (ST + GRP - 1) // GRP  # v-cache DMA groups per batch

    with (
        tc.tile_pool(name="vslab", bufs=VG + 1) as vslab_pool,
        tc.tile_pool(name="ktslab", bufs=3) as kt_pool,
        tc.tile_pool(name="pexp", bufs=6) as pexp_pool,
        tc.tile_pool(name="attn_sm", bufs=2) as attn_sm,
        tc.tile_pool(name="acc_pool", bufs=2) as acc_pool,
        tc.tile_pool(name="outT", bufs=2) as outT_pool,
        tc.tile_pool(name="ysb", bufs=3) as ysb_pool,
        tc.tile_pool(name="attn_psum", bufs=2, space="PSUM") as attn_psum,
    ):
        outT_all = outT_pool.tile([128, HPC, TOK], DT, tag="outT", name="outT_all")
        yT_g = _grouped_rows(t["yT"])

        # k-cache loads ride the SP ring (idle after the projection loads);
        # issued one head ahead of use through the 2-slot pool
        kt_order = [(b, h) for b in range(B) for h in range(HPC)]
        kt_slabs = {}

        def _issue_kt(i):
            if i >= len(kt_order):
                return
            bb, hh = kt_order[i]
            s = kt_pool.tile([Dh, KEEP], DT, tag="kt", name=f"kt{bb}{hh}")
            nc.sync.dma_start(s[:], t["kTc"][bb, hh])
            kt_slabs[(bb, hh)] = s

        _issue_kt(0)

        if OPT_GATE:
            # Gate the Pool-ring cache traffic behind K-proj head 0: the Pool
            # queue is FIFO, so this copy (which depends on kn_sb[0]) delays
            # the v-cache/wo DMAs until ~1/3 into the projections, keeping the
            # early shared-HBM pipe exclusively on the PE weight stream.
            gate = attn_sm.tile([1, 1], DT, tag="gate", name="poolgate")
            nc.gpsimd.tensor_copy(gate[:], kn_sb[0][0:1, 0:1])

        # all v-cache loads up front on the Pool SWDGE ring (Pool is idle);
        # SBUF slots for the tail of b=1 rotate as b=0 groups release
        v_groups_all = []
        for b in range(B):
            vgs = []
            for g in range(VG):
                j0 = g * GRP
                jn = min(GRP, ST - j0)
                vg = vslab_pool.tile([128, GRP, HPC * Dh], DT, tag="v", name=f"v{b}_{g}")
                src = t["vc"][b, j0 * 128 : (j0 + jn) * 128, :].rearrange(
                    "(j p) f -> p j f", p=128
                )
                nc.gpsimd.dma_start(vg[:, :jn, :], src)
                vgs.append(vg)
            v_groups_all.append(vgs)
        # wo after the v-cache on the Pool ring; needed only for the b=1
        # output-projection interleave (~60% into the kernel)
        nc.gpsimd.dma_start(wo_sb[:], t["wo"][:])

        # b=1 V projection (c=2,3), PSUM in the yps slots (unused until the
        # b=1 output projection), reading the SBUF-resident wv slabs
        v23_units = []
        if not DIAG_NO_QKV:
            vps23 = {
                c: attn_psum.tile([128, HPC * Dh], F32, tag="yps", name=f"vps{c}")
                for c in (2, 3)
            }

            def _mk_v23_unit(g):
                def _unit():
                    slab = wv_slabs[g]
                    for j in range(GRP):
                        kt = g * GRP + j
                        for c in (2, 3):
                            nc.tensor.matmul(
                                vps23[c][:],
                                xt_tile(kt)[:, c * 128 : (c + 1) * 128],
                                slab[:, j, :],
                                start=(kt == 0),
                                stop=(kt == KT - 1),
                            )
                return _unit

            def _v23_copies():
                for c in (2, 3):
                    nc.vector.tensor_copy(vnew_sb[:, c, :], vps23[c][:])

            v23_units = [_mk_v23_unit(g) for g in range(KT // GRP)]
            v23_units.append(_v23_copies)

        def emit_head(b, h, filler=None):
            """Attention for one (batch, head): fills outT_all[:, h, b*L:]."""
            v_groups = v_groups_all[b]
            idx = kt_order.index((b, h))
            _issue_kt(idx + 1)
            kt_slab = kt_slabs[(b, h)]

            if DIAG_NO_ATTN:
                nc.vector.memset(outT_all[:, h, b * L : (b + 1) * L], 0.01)
                return
            # one PSUM bank per head: lane 0 accumulates attn@V, lane 1 hosts
            # the softmax-sum row and then the broadcast reciprocal (the bank
            # would otherwise be half-wasted by 2KB rounding)
            ot = attn_psum.tile([128, 2, L], F32, tag="oacc", name=f"oacc{b}{h}")
            out_ps = ot[:, 0, :]
            q_rhs = qn_sb[h][:, b * L : (b + 1) * L]

            NL = 4 if OPT_ACC4 else 2
            acc = acc_pool.tile([128, NL, L], DT, tag="acc", name=f"acc{b}{h}")
            n_chunks = ST + 2
            chunk_idx = 0
            pending = []  # deferred attnV emissions (up to two groups behind)
            DEPTH = 2 if OPT_DELAY_ATTNV else 0
            for gi, (kind, chunks) in enumerate(groups):
                w = len(chunks)
                lg = attn_psum.tile([128, GRPA, L], F32, tag="lg", name=f"lg{b}{h}{gi}")
                for j, cidx in enumerate(chunks):
                    if kind == "old":
                        lhsT = kt_slab[:, cidx * 128 : (cidx + 1) * 128]
                    else:
                        lhsT = kn_sb[h][:, b * L + cidx * 128 : b * L + (cidx + 1) * 128]
                    nc.tensor.matmul(lg[:, j, :], lhsT, q_rhs, start=True, stop=True)
                pexp = pexp_pool.tile([128, GRPA, L], DT, tag="pexp", name=f"pe{b}{h}{gi}")
                nc.scalar.activation(
                    pexp[:, :w, :],
                    lg[:, :w, :],
                    mybir.ActivationFunctionType.Exp,
                    scale=SCALE,
                )
                if kind == "new":
                    nc.vector.tensor_mul(pexp[:, :2, :], pexp[:, :2, :], mask_sb[:])
                if not DIAG_NO_SUMS:
                    if OPT_ACC4:
                        # four acc lanes, one wide add per group: half the
                        # DVE instructions/semaphores of the 2-lane scheme,
                        # and the serial chain still beats the ACT exp pace
                        if gi == 0:
                            nc.vector.tensor_copy(acc[:, :w, :], pexp[:, :w, :])
                        else:
                            nc.vector.tensor_add(
                                acc[:, :w, :], acc[:, :w, :], pexp[:, :w, :]
                            )
                    else:
                        if gi == 0:
                            nc.vector.tensor_copy(acc[:, :2, :], pexp[:, :2, :])
                            lo = 2
                        else:
                            lo = 0
                        for c0 in range(lo, w, 2):
                            cw = min(2, w - c0)
                            nc.vector.tensor_add(
                                acc[:, :cw, :],
                                acc[:, :cw, :],
                                pexp[:, c0 : c0 + cw, :],
                            )
                # Emit attn@V one group LATE: when the PE's in-order FIFO
                # reaches attnV(g-1), exp(g-1) already finished during lg(g),
                # so the PE never head-of-line blocks on the ACT engine.
                def _attnv(kind, chunks, pexp, ci0):
                    cix = ci0
                    for j, cidx in enumerate(chunks):
                        first = cix == 0
                        last = cix == n_chunks - 1
                        if kind == "old":
                            v_lhsT = v_groups[cidx // GRP][:, cidx % GRP, h * Dh : (h + 1) * Dh]
                        else:
                            v_lhsT = vnew_sb[:, b * 2 + cidx, h * Dh : (h + 1) * Dh]
                        nc.tensor.matmul(
                            out_ps, v_lhsT, pexp[:, j, :], start=first, stop=last
                        )
                        cix += 1

                pending.append((kind, chunks, pexp, chunk_idx))
                chunk_idx += len(chunks)
                while len(pending) > DEPTH:
                    _attnv(*pending.pop(0))
                # drip-feed interleaved PE work (b=1 V projection) into the
                # ACT-bound stretches of b=0's attention
                if filler and gi % 4 == 3:
                    filler.pop(0)()
            while pending:
                _attnv(*pending.pop(0))
            if DIAG_NO_SUMS:
                nc.vector.tensor_copy(outT_all[:, h, b * L : (b + 1) * L], out_ps)
                return
            # softmax denominator: fold the acc lanes on PE into lane 1 of the
            # oacc bank, reciprocal on DVE, broadcast back through PE, then
            # one fused normalize multiply into outT
            sum_ps = ot[0:1, 1, :]
            for ln in range(NL):
                nc.tensor.matmul(
                    sum_ps, ones_col16[:], acc[:, ln, :],
                    start=(ln == 0), stop=(ln == NL - 1),
                )
            recip32 = attn_sm.tile([1, L], F32, tag="recip32", name=f"rc32{b}{h}")
            nc.vector.reciprocal_approx_fast(recip32[:], sum_ps)
            recip = attn_sm.tile([1, L], DT, tag="recip", name=f"rc{b}{h}")
            nc.vector.tensor_copy(recip[:], recip32[:])
            bc_ps = ot[:, 1, :]
            nc.tensor.matmul(bc_ps, ones_row16[:], recip[:], start=True, stop=True)
            ocopy = attn_sm.tile([128, L], F32, tag="ocopy", name=f"oc{b}{h}")
            nc.vector.tensor_copy(ocopy[:], out_ps)
            nc.vector.tensor_mul(
                outT_all[:, h, b * L : (b + 1) * L], ocopy[:], bc_ps
            )

        ncopy = [0]

        def emit_outproj_group(g, half):
            """Output projection for m-tiles [4g, 4g+4) over token half `half`."""
            y_sb = ysb_pool.tile([128, GRP, L], DT, tag="ysb", name=f"y{half}_{g}")
            for jj in range(GRP // 2):
                m0 = g * GRP + 2 * jj
                yps = attn_psum.tile([128, 2, L], F32, tag="yps", name=f"yp{half}_{m0}")
                for j in range(2):
                    m = m0 + j
                    for h in range(HPC):
                        nc.tensor.matmul(
                            yps[:, j, :],
                            wo_sb[:, h, m * 128 : (m + 1) * 128],
                            outT_all[:, h, half * L : (half + 1) * L],
                            start=(h == 0),
                            stop=(h == HPC - 1),
                        )
                dst = y_sb[:, 2 * jj : 2 * jj + 2, :]
                if ncopy[0] % 2 == 0:
                    nc.vector.tensor_copy(dst, yps[:])
                else:
                    nc.scalar.copy(dst, yps[:])
                ncopy[0] += 1
            # stores ride SP (idle) / ACT alternately
            dstg = yT_g[g][:, :, half * L : (half + 1) * L]
            if g % 2 == 0:
                nc.sync.dma_start(dstg, y_sb[:])
            else:
                nc.scalar.dma_start(dstg, y_sb[:])

        # b=0 attention (with b=1's V projection drip-fed in); then b=1
        # attention with b=0's output projection interleaved (keeps PE busy
        # while ACT works through the exps); b=1's output projection is the
        # tail.
        fill = list(v23_units)
        for h in range(HPC):
            emit_head(0, h, fill)
        for u in fill:
            u()
        fill.clear()
        for h in range(HPC):
            emit_head(1, h)
            emit_outproj_group(2 * h, 0)
            emit_outproj_group(2 * h + 1, 0)
        for g in range(KT // GRP):
            emit_outproj_group(g, 1)


def build_program(reps_loop=False, unroll=1):
    nc = bacc.Bacc(
        "TRN2",
        target_bir_lowering=False,
        debug=False,
        enable_asserts=False,
        num_devices=N_CORES,
    )
    t = _declare_io(nc, with_reps=reps_loop)

    with (
        nc.allow_low_precision(reason="deliberate fp16/fp32r compute"),
        tile.TileContext(nc) as tc,
    ):
        with contextlib.ExitStack() as ctx:
            consts_pool = ctx.enter_context(tc.tile_pool(name="consts", bufs=1))
            xt_pool = ctx.enter_context(tc.tile_pool(name="xt", bufs=1))
            wo_pool = ctx.enter_context(tc.tile_pool(name="wop", bufs=1))
            qkv_pool = ctx.enter_context(tc.tile_pool(name="qkv", bufs=1))
            wv_pool = ctx.enter_context(tc.tile_pool(name="wvp", bufs=8))

            qw_ld = consts_pool.tile([1, Dh], F32)
            nc.sync.dma_start(qw_ld[:], t["qw"][:])
            qw_sb = consts_pool.tile([1, Dh], DT)
            nc.vector.tensor_copy(qw_sb[:], qw_ld[:])
            kw_ld = consts_pool.tile([1, Dh], F32)
            nc.sync.dma_start(kw_ld[:], t["kw"][:])
            kw_sb = consts_pool.tile([1, Dh], DT)
            nc.vector.tensor_copy(kw_sb[:], kw_ld[:])
            mask_sb = consts_pool.tile([128, 2, L], DT)
            nc.sync.dma_start(mask_sb[:], t["maskt"][:])
            ones_col16 = consts_pool.tile([128, 1], DT)
            nc.vector.memset(ones_col16[:], 1.0)
            ones_row16 = consts_pool.tile([1, 128], DT)
            nc.vector.memset(ones_row16[:], 1.0)
            eps_sb = consts_pool.tile([1, 1], F32)
            nc.vector.memset(eps_sb[:], EPS)

            consts = (qw_sb, kw_sb, mask_sb, ones_col16, ones_row16, eps_sb)
            pools = {"qkv": qkv_pool, "xt": xt_pool, "wo": wo_pool, "wv": wv_pool}

            if reps_loop:
                reps_sb = consts_pool.tile([1, 1], mybir.dt.uint32)
                nc.sync.dma_start(reps_sb[:], t["reps"][:])
                reps_regs = nc.alloc_registers("reps_regs")
                nc.regs_load(reps_regs, reps_sb[0:1, 0:1])
                reps_val = nc.snap(reps_regs, donate=True, min_val=1, max_val=1 << 20)
                with tc.For_i(0, reps_val, 1):
                    for _ in range(unroll):
                        _emit_body(nc, tc, t, consts, pools)
            else:
                for _ in range(unroll):
                    _emit_body(nc, tc, t, consts, pools)

    nc.compile()
    return nc


def prep_inputs(x, cache_k, cache_v, Wq, Wk, Wv, Wo, q_norm_w, k_norm_w, sink):
    """Host-side sharding/layout prep. Returns in_maps for the 8 cores."""
    x = np.asarray(x, dtype=np.float32)
    cache_k = np.asarray(cache_k, dtype=np.float32)
    cache_v = np.asarray(cache_v, dtype=np.float32)
    Wq = np.asarray(Wq, dtype=np.float32)
    Wk = np.asarray(Wk, dtype=np.float32)
    Wv = np.asarray(Wv, dtype=np.float32)
    Wo = np.asarray(Wo, dtype=np.float32)
    q_norm_w = np.asarray(q_norm_w, dtype=np.float32)
    k_norm_w = np.asarray(k_norm_w, dtype=np.float32)
    sink = int(sink)
    assert sink == SINK, f"kernel hardcodes sink={SINK}, got {sink}"

    xT = np.ascontiguousarray(x.reshape(TOK, D).T.astype(DT_NP))  # [D, TOK]

    # kept old-cache rows: [0:sink] ++ [sink+L : S]
    ck = np.concatenate([cache_k[:, :SINK], cache_k[:, SINK + L :]], axis=1)
    cv = np.concatenate([cache_v[:, :SINK], cache_v[:, SINK + L :]], axis=1)

    qw = np.ascontiguousarray(q_norm_w.reshape(1, Dh))
    kw = np.ascontiguousarray(k_norm_w.reshape(1, Dh))

    # mask[p, c, l] = 1 if new-chunk position c*128+p is visible to query l
    pi = np.arange(128)[:, None, None]
    cc = np.arange(2)[None, :, None]
    ll = np.arange(L)[None, None, :]
    maskt = ((cc * 128 + pi) <= ll).astype(DT_NP)

    in_maps = []
    for core in range(N_CORES):
        hs = core * HPC
        fs = hs * Dh  # feature start column
        fe = fs + HPC * Dh
        wq_c = np.ascontiguousarray(Wq[:, fs:fe].astype(DT_NP))
        wk_c = np.ascontiguousarray(Wk[:, fs:fe].astype(DT_NP))
        wv_c = np.ascontiguousarray(Wv[:, fs:fe].astype(DT_NP))
        wo_c = np.ascontiguousarray(
            Wo[fs:fe, :].reshape(HPC, 128, D).transpose(1, 0, 2).astype(DT_NP)
        )  # [128, HPC, D]
        kTc = np.ascontiguousarray(
            ck[:, :, hs : hs + HPC, :].transpose(0, 2, 3, 1).astype(DT_NP)
        )  # [B, HPC, Dh, KEEP]
        vc = np.ascontiguousarray(
            cv[:, :, hs : hs + HPC, :].reshape(B, KEEP, HPC * Dh).astype(DT_NP)
        )  # [B, KEEP, HPC*Dh]
        in_maps.append(
            {
                "xT": xT,
                "wq": wq_c,
                "wk": wk_c,
                "wv": wv_c,
                "wo": wo_c,
                "kTc": kTc,
                "vc": vc,
                "qw": qw,
                "kw": kw,
                "maskt": maskt,
            }
        )
    return in_maps


_PROGRAM_CACHE = {}


def _get_program(reps_loop=False):
    key = bool(reps_loop)
    if key not in _PROGRAM_CACHE:
        _PROGRAM_CACHE[key] = build_program(reps_loop=key)
    return _PROGRAM_CACHE[key]


def kernel(**inputs) -> np.ndarray:
    in_maps = prep_inputs(**inputs)
    nc = _get_program(reps_loop=False)
    res = run_bass_kernel_spmd(nc, in_maps, core_ids=list(range(N_CORES)))
    yT = np.zeros((D, TOK), np.float64)
    for c in range(N_CORES):
        yT += res.results[c]["yT"].astype(np.float64)  # fp16 partials, fp64 sum
    y = yT.T.reshape(B, L, D).astype(np.float32)
    return y


if __name__ == "__main__":
    rng = np.random.default_rng(0)
    inputs = {
        "x": rng.standard_normal((B, L, D), dtype=np.float32),
        "cache_k": rng.standard_normal((B, S, H, Dh), dtype=np.float32),
        "cache_v": rng.standard_normal((B, S, H, Dh), dtype=np.float32),
        "Wq": (rng.standard_normal((D, D), dtype=np.float32) * 0.02),
        "Wk": (rng.standard_normal((D, D), dtype=np.float32) * 0.02),
        "Wv": (rng.standard_normal((D, D), dtype=np.float32) * 0.02),
        "Wo": (rng.standard_normal((D, D), dtype=np.float32) * 0.02),
        "q_norm_w": np.ones(Dh, np.float32),
        "k_norm_w": np.ones(Dh, np.float32),
        "sink": SINK,
    }
    y = kernel(**inputs)
    print("y", y.shape, y.dtype, float(np.abs(y).mean()))

